# revision 1
# baseline (speedup 1.0000x reference)
"""nn_CAM_Module kernel for 8 Trainium2 NeuronCores (Bass/Tile).

Contract: kernel(**inputs) takes the FULL inputs (x: [16, 512, 64, 64] fp32,
gamma: [1] fp32) and returns the FULL output, sharding batch B=16 across the
8 cores (2 samples per core, gamma replicated) — per the data-parallel
sharding: every op is a per-sample bmm, no cross-core communication.

Per-sample computation (C=512 channels, N=H*W=4096):
  energy = xf @ xf.T                          (C,C), contraction over N on PE
  m_i    = min_j energy[i,j]                  (softmax(max-e) == softmax(m-e))
  P_ij   = exp(m_i - energy_ij), S_i = sum_j  (ACT, fused row-sum)
  out    = diag(1/S) @ (P @ xf)               (PE; P^T tiles via PE transpose)
  y      = gamma * out + x                    (fused DVE mult-add)

Layouts per core (P=128 partitions):
  xf   [128, 4, 4096] fp32   channel blocks on partitions (DMA from DRAM)
  xfc  [128, 4, 4096] mm_dt  low-precision cast (matmul operand)
  xfT  [128, 32, 512] mm_dt  spatial chunks on partitions (PE transposes)
  Pmat [128, 4, 512]  mm_dt  attention numerator, rows i
  PT   [128, 4, 512]  mm_dt  P^T tiles (PE transposes), matmul2 stationary
"""

import os
from contextlib import ExitStack

import numpy as np

B, C, H, W = 16, 512, 64, 64
N = H * W
N_CORES = 8
BPC = B // N_CORES
P = 128

MM_DT_NAME = os.environ.get("CAM_MM_DT", "fp8")

LAST_EXEC_TIME_NS = None
LAST_TRACE = None
LAST_PROFILE_JSON = None
_CACHE = {}


def _build(mm_dt_name):
    import concourse.mybir as mybir
    import concourse.tile as tile
    from concourse import bacc
    from concourse.masks import make_identity

    F32 = mybir.dt.float32
    mm_dt = {
        "bf16": mybir.dt.bfloat16,
        "fp8": mybir.dt.float8e4,
        "f32": F32,
    }[mm_dt_name]
    DR = mm_dt in (mybir.dt.float8e4, mybir.dt.float8e5)

    CB = C // P          # 4 channel blocks
    KB = N // P          # 32 spatial chunks
    NCH_SZ = 512
    NCH = N // NCH_SZ    # 8 output column chunks

    nc = bacc.Bacc(None, target_bir_lowering=False, debug=False)
    x = nc.dram_tensor("x", [BPC, C, N], F32, kind="ExternalInput")
    gamma = nc.dram_tensor("gamma", [1], F32, kind="ExternalInput")
    y = nc.dram_tensor("y", [BPC, C, N], F32, kind="ExternalOutput")

    with ExitStack() as ctx:
        tc = ctx.enter_context(tile.TileContext(nc))
        singles = ctx.enter_context(tc.tile_pool(name="singles", bufs=1))
        xf_pool = ctx.enter_context(tc.tile_pool(name="xf", bufs=12))
        xfc_pool = ctx.enter_context(tc.tile_pool(name="xfc", bufs=12))
        xfT_pool = ctx.enter_context(tc.tile_pool(name="xfT", bufs=2))
        pmat_pool = ctx.enter_context(tc.tile_pool(name="pmat", bufs=2))
        pt_pool = ctx.enter_context(tc.tile_pool(name="pt", bufs=2))
        small = ctx.enter_context(tc.tile_pool(name="small", bufs=16))
        yt_pool = ctx.enter_context(tc.tile_pool(name="yt", bufs=3))
        eps_pool = ctx.enter_context(tc.tile_pool(name="eps", bufs=4, space="PSUM"))
        tps_pool = ctx.enter_context(tc.tile_pool(name="tps", bufs=2, space="PSUM"))
        ops_pool = ctx.enter_context(tc.tile_pool(name="ops", bufs=2, space="PSUM"))

        ident = singles.tile([P, P], mm_dt)
        make_identity(nc, ident)
        gamma_sb = singles.tile([P, 1], F32)
        nc.sync.dma_start(gamma_sb[:], gamma[:].to_broadcast((P, 1)))

        # ~3.5us of dummy matmuls while the first chunk loads: warms the
        # PE HAM clock-gate (transpose-mode work doesn't), so the first
        # real transposes run at 2.4GHz instead of 1.2.
        warm_src = singles.tile([P, 512], mm_dt)
        nc.vector.memset(warm_src[:], 0.0)
        warm_ps = ops_pool.tile([P, NCH_SZ], F32, tag="ops", name="warm_ps")
        for w in range(16):
            nc.tensor.matmul(
                warm_ps[:], ident[:], warm_src[:],
                start=(w == 0), stop=(w == 15),
            )

        # fp8 PE-transpose writes PSUM with element step 2 (16-bit write
        # packing): stage into a 2x-strided PSUM view, copy back strided.
        TW = 2 if DR else 1
        KPC = NCH_SZ // P  # transposes-k per n-chunk

        def tps_views(tps):
            if TW == 1:
                return tps, tps
            v = tps[:].rearrange("p cb (n t) -> p cb n t", t=TW)[:, :, :, 0]
            return v, v

        # ---- software pipeline over samples ----
        # prefetch_chunk(b, ch): load 1MB n-chunk, cast, PE-transpose into
        #   xfT, and accumulate this chunk's k-pairs into the energy PSUMs.
        # softmax(b): row-min + exp(+rowsum) + beta + P^T tiles.
        # mm2_chunk(b, nh): attention matmul + fused epilogue + y write.
        # Emission interleaves sample b's mm2 chunks with sample b+1's
        # prefetch chunks so neither PE nor DMA drains between samples.
        states = {}

        def load_chunk(b, ch):
            """DMA-only part: issue the 1MB chunk load (sync queue). Safe to
            hoist ahead of the previous sample's softmax/mm2 emission — it
            adds no PE/DVE/ACT work there, just keeps the DMA engines fed."""
            st = states.setdefault(b, {"xf": [], "xfc": []})
            if len(st["xf"]) > ch:
                return
            xv = x[b].rearrange("(cb p) n -> p cb n", p=P)
            nsl = slice(ch * NCH_SZ, (ch + 1) * NCH_SZ)
            xfch = xf_pool.tile([P, CB, NCH_SZ], F32, tag="xf", name=f"xf{b}_{ch}")
            if b == 0 and ch == 0:
                # split the very first load per-cb so the first cast (and
                # with it the first PE transpose) starts ~3us earlier
                for cb in range(CB):
                    nc.sync.dma_start(xfch[:, cb, :], xv[:, cb, nsl])
            else:
                nc.sync.dma_start(xfch[:], xv[:, :, nsl])
            st["xf"].append(xfch)

        def prefetch_chunk(b, ch):
            load_chunk(b, ch)
            st = states[b]
            if "xfT" not in st:
                st["xfT"] = xfT_pool.tile([P, KB, C], mm_dt, tag="xfT", name=f"xfT{b}")
                st["eps"] = [
                    eps_pool.tile([P, C], F32, tag="eps", name=f"eps{b}_{i}")
                    for i in range(CB)
                ]
            xfch = st["xf"][ch]
            xfcch = xfc_pool.tile([P, CB, NCH_SZ], mm_dt, tag="xfc")
            # fine-grained per-cb casts so the first transposes start
            # right after the first sub-cast, split across DVE/ACT
            for cb in range(CB):
                # 3:1 toward ACT: DVE is the busier engine (epilogue+copies)
                if (ch * CB + cb) % 4 == 0:
                    nc.vector.tensor_copy(out=xfcch[:, cb, :], in_=xfch[:, cb, :])
                else:
                    nc.scalar.copy(out=xfcch[:, cb, :], in_=xfch[:, cb, :])
            st["xfc"].append(xfcch)
            xfT = st["xfT"]
            # two k-groups share one PSUM bank: 8 transposes, one copy
            for kk in range(0, KPC, 2):
                k = ch * KPC + kk
                tps = tps_pool.tile([P, 2, CB, P * TW], mm_dt, tag="tps")
                if TW == 1:
                    wv = tps[:]
                else:
                    wv = tps[:].rearrange("p u cb (n t) -> p u cb n t", t=TW)[
                        :, :, :, :, 0
                    ]
                for u in range(2):
                    for cb in range(CB):
                        nc.tensor.transpose(
                            wv[:, u, cb, :],
                            xfcch[:, cb, (kk + u) * P : (kk + u + 1) * P],
                            ident,
                        )
                dst = xfT[:, k : k + 2, :].rearrange("p u (cb n) -> p u cb n", n=P)
                # xfT copies mostly on ACT to unload DVE
                if (ch * 2 + kk // 2) % 4 == 0:
                    nc.vector.tensor_copy(out=dst, in_=wv)
                else:
                    nc.scalar.copy(out=dst, in_=wv)
            # energy accumulation for this chunk's k-pairs
            for cb in range(CB):
                e_ps = st["eps"][cb]
                if DR:
                    for kk in range(0, KPC, 2):
                        k = ch * KPC + kk
                        nc.tensor.matmul(
                            e_ps[:],
                            xfT[:, k : k + 2, cb * P : (cb + 1) * P],
                            xfT[:, k : k + 2, :],
                            start=(k == 0),
                            stop=(k + 2 >= KB),
                            perf_mode=mybir.MatmulPerfMode.DoubleRow,
                        )
                else:
                    for kk in range(KPC):
                        k = ch * KPC + kk
                        nc.tensor.matmul(
                            e_ps[:],
                            xfT[:, k, cb * P : (cb + 1) * P],
                            xfT[:, k, :],
                            start=(k == 0),
                            stop=(k == KB - 1),
                        )

        def softmax(b):
            st = states[b]
            Pmat = pmat_pool.tile([P, CB, C], mm_dt, tag="pmat")
            rS = small.tile([P, CB], F32, tag="rS")
            for cb in range(CB):
                e_ps = st["eps"][cb]
                m = small.tile([P, 1], F32, tag="m")
                nc.vector.tensor_reduce(
                    out=m[:], in_=e_ps[:], axis=mybir.AxisListType.X,
                    op=mybir.AluOpType.min,
                )
                S = small.tile([P, 1], F32, tag="S")
                nc.scalar.activation(
                    out=Pmat[:, cb, :],
                    in_=e_ps[:],
                    func=mybir.ActivationFunctionType.Exp,
                    bias=m[:],
                    scale=-1.0,
                    accum_out=S[:],
                )
                nc.vector.reciprocal(out=rS[:, cb : cb + 1], in_=S[:])

            beta = small.tile([P, CB], F32, tag="beta")
            nc.vector.tensor_tensor(
                out=beta[:],
                in0=rS[:],
                in1=gamma_sb[:].to_broadcast((P, CB)),
                op=mybir.AluOpType.mult,
            )
            st["beta"] = beta

            # PT transposes grouped by source row-block ob so each group can
            # start as soon as exp(ob) lands (no wait for all four exps).
            PT = pt_pool.tile([P, CB, C], mm_dt, tag="pt")
            for ob in range(CB):
                tps = tps_pool.tile([P, CB, P * TW], mm_dt, tag="tps")
                wv, rv = tps_views(tps)
                for cb in range(CB):
                    nc.tensor.transpose(
                        wv[:, cb, :], Pmat[:, ob, cb * P : (cb + 1) * P], ident
                    )
                dst = PT[:, :, ob * P : (ob + 1) * P]
                if ob % 2 == 0:
                    nc.vector.tensor_copy(out=dst, in_=rv)
                else:
                    nc.scalar.copy(out=dst, in_=rv)
            st["PT"] = PT

        def mm2_chunk(b, nh):
            st = states[b]
            PT, beta = st["PT"], st["beta"]
            yv = y[b].rearrange("(ob p) n -> p ob n", p=P)
            nsl = slice(nh * NCH_SZ, (nh + 1) * NCH_SZ)
            yt = yt_pool.tile([P, CB, NCH_SZ], F32, tag="yt")
            for ob in range(CB):
                o_ps = ops_pool.tile([P, NCH_SZ], F32, tag="ops")
                if DR:
                    for cb in range(0, CB, 2):
                        nc.tensor.matmul(
                            o_ps[:],
                            PT[:, cb : cb + 2, ob * P : (ob + 1) * P],
                            st["xfc"][nh][:, cb : cb + 2, :],
                            start=(cb == 0),
                            stop=(cb + 2 >= CB),
                            perf_mode=mybir.MatmulPerfMode.DoubleRow,
                        )
                else:
                    for cb in range(CB):
                        nc.tensor.matmul(
                            o_ps[:],
                            PT[:, cb, ob * P : (ob + 1) * P],
                            st["xfc"][nh][:, cb, :],
                            start=(cb == 0),
                            stop=(cb == CB - 1),
                        )
                nc.vector.scalar_tensor_tensor(
                    out=yt[:, ob, :],
                    in0=o_ps[:],
                    scalar=beta[:, ob : ob + 1],
                    in1=st["xf"][nh][:, ob, :],
                    op0=mybir.AluOpType.mult,
                    op1=mybir.AluOpType.add,
                )
            # SWDGE so writes don't block the next sample's loads in the
            # HWDGE FIFO (gpsimd engine is otherwise idle)
            nc.gpsimd.dma_start(yv[:, :, nsl], yt[:])

        for ch in range(NCH):
            prefetch_chunk(0, ch)
        for b in range(BPC):
            if b + 1 < BPC:
                # hoist the next sample's first loads (DMA only) so they
                # queue right behind this sample's loads on the sync FIFO
                for ch in range(min(4, NCH)):
                    load_chunk(b + 1, ch)
            softmax(b)
            for nh in range(NCH):
                mm2_chunk(b, nh)
                if b + 1 < BPC:
                    prefetch_chunk(b + 1, nh)

    nc.finalize()
    return nc


def kernel(x: np.ndarray, gamma: np.ndarray) -> np.ndarray:
    global LAST_EXEC_TIME_NS, LAST_TRACE, LAST_PROFILE_JSON
    from concourse.bass_utils import run_bass_kernel_spmd

    assert x.shape == (B, C, H, W), x.shape
    x = np.ascontiguousarray(x, dtype=np.float32)
    gamma = np.ascontiguousarray(gamma, dtype=np.float32).reshape(1)

    name = MM_DT_NAME
    if name not in _CACHE:
        _CACHE[name] = _build(name)
    nc = _CACHE[name]

    xs = x.reshape(N_CORES, BPC, C, N)
    in_maps = [{"x": xs[i], "gamma": gamma} for i in range(N_CORES)]
    trace = os.environ.get("CAM_TRACE", "0") == "1"
    kwargs = {}
    if trace:
        import tempfile

        tmpdir = tempfile.mkdtemp(prefix=f"cam_trace_{name}_")
        try:
            os.unlink(f"/tmp/cam_trace_{name}")
        except OSError:
            pass
        os.symlink(tmpdir, f"/tmp/cam_trace_{name}")
        kwargs["tmpdir"] = tmpdir
    res = run_bass_kernel_spmd(
        nc, in_maps, core_ids=list(range(N_CORES)), trace=trace, **kwargs
    )
    LAST_EXEC_TIME_NS = res.exec_time_ns
    LAST_TRACE = res.instructions_and_trace
    LAST_PROFILE_JSON = res.profile_json
    out = np.concatenate([res.results[i]["y"] for i in range(N_CORES)], axis=0)
    return out.reshape(B, C, H, W)



# revision 2
# speedup vs baseline: 1.0991x; 1.0991x over previous
"""nn_CAM_Module kernel for 8 Trainium2 NeuronCores (Bass/Tile).

Contract: kernel(x: [16, 512, 64, 64] f32, gamma: [1] f32) -> full [16, 512,
64, 64] f32 output. Batch is sharded 2 samples/core across 8 cores, gamma
replicated (every op is a per-sample bmm, no cross-core communication).

Design (vs the chunked fp32 v1):
 - sigma channel layout: partition p holds channels {4p+j, j=0..3}, so DRAM
   reads are 4KB-contiguous per descriptor and y writes 8KB-contiguous.
 - loads are SWDGE DMA-casts fp32->bf16: x only ever lands in SBUF as bf16
   (the +x epilogue term and the fp8 matmul operands both derive from it),
   removing a full-sample engine cast. The bf16 rounding of the output is
   ~0.4% rel err, well inside the 2e-2 gate.
 - y is written as bf16 (halves HBM write traffic); the host upcasts to f32.
 - the softmax numerator is pre-scaled by beta = gamma/rowsum, so mm2
   produces beta*(P@x) directly. The +x is either added by the DVE epilogue
   op (which doubles as the PSUM->SBUF move) or, for a fraction of chunks,
   on the PE via an identity-bf16 matmul into the same PSUM accumulation
   with a pure copy epilogue on ACT - balancing PE/DVE/ACT load.
 - mm2 runs output-block-major: each y row-block (1MB) DMAs out as soon as
   its 8 PSUM chunks finish, in place over the bf16 x buffer.

Per-sample math (C=512, N=4096; m = j*128+p <-> c = 4p+j permutation; the
row softmax is permutation-invariant and inputs/outputs are permuted
consistently):
  energy = xf @ xf.T          (fp8 DoubleRow matmuls over PE-transposed tiles)
  P_ij   = beta_i * exp(min_j E_ij - E_ij),  beta = gamma / rowsum
  y      = P @ xf + x         (fp8 DR + epilogue add, written back as bf16)
"""

import os
from contextlib import ExitStack

import numpy as np

B, C, H, W = 16, 512, 64, 64
N = H * W
N_CORES = 8
BPC = B // N_CORES
P = 128
J = 4                 # channels per partition: c = 4p + j
NCH = 4               # load chunks per sample
NW = N // NCH         # 1024 spatial positions per chunk
KB = N // P           # 32 transpose blocks
KPC = NW // P         # 8 transpose blocks per chunk
NHC = 8               # mm2 psum chunks (512 wide)
NHW = N // NHC

LAST_EXEC_TIME_NS = None
LAST_TRACE = None
LAST_PROFILE_JSON = None
MM_DT_NAME = "fp8"    # informational; the kernel is fp8-DR + bf16 I/O
_CACHE = {}


def _ensure_ntff_hook():
    """Register the axon NTFF profile hook if the environment lacks
    antenv.axon_hooks (needed only when tracing; harmless otherwise)."""
    import sys
    import types

    try:
        from antenv.axon_hooks import get_axon_ntff_profile_hook  # noqa: F401
        return
    except ImportError:
        pass
    try:
        import antenv
        from trn_agent_boot.trn_boot import _ntff_profile_via_ctypes

        mod = types.ModuleType("antenv.axon_hooks")
        _hook = [None]
        mod.set_axon_ntff_profile_hook = lambda h: _hook.__setitem__(0, h)
        mod.get_axon_ntff_profile_hook = lambda: _hook[0]
        antenv.axon_hooks = mod
        sys.modules["antenv.axon_hooks"] = mod
        mod.set_axon_ntff_profile_hook(
            _ntff_profile_via_ctypes("/opt/axon/libaxon_pjrt.so")
        )
    except Exception:
        pass


def _build():
    import concourse.mybir as mybir
    import concourse.tile as tile
    from concourse import bacc
    from concourse.masks import make_identity

    F32 = mybir.dt.float32
    BF16 = mybir.dt.bfloat16
    FP8 = mybir.dt.float8e4
    DR = mybir.MatmulPerfMode.DoubleRow

    nc = bacc.Bacc(None, target_bir_lowering=False, debug=False)
    x = nc.dram_tensor("x", [BPC, C, N], F32, kind="ExternalInput")
    gamma = nc.dram_tensor("gamma", [1], F32, kind="ExternalInput")
    y = nc.dram_tensor("y", [BPC, C, N], BF16, kind="ExternalOutput")

    with ExitStack() as ctx:
        tc = ctx.enter_context(tile.TileContext(nc))
        singles = ctx.enter_context(tc.tile_pool(name="singles", bufs=1))
        xb_pool = ctx.enter_context(tc.tile_pool(name="xb", bufs=2))
        xfc_pool = ctx.enter_context(tc.tile_pool(name="xfc", bufs=2))
        xfT_pool = ctx.enter_context(tc.tile_pool(name="xfT", bufs=2))
        pmat_pool = ctx.enter_context(tc.tile_pool(name="pmat", bufs=2))
        pt_pool = ctx.enter_context(tc.tile_pool(name="pt", bufs=2))
        small = ctx.enter_context(tc.tile_pool(name="small", bufs=16))
        eps_pool = ctx.enter_context(tc.tile_pool(name="eps", bufs=4, space="PSUM"))
        tps_pool = ctx.enter_context(tc.tile_pool(name="tps", bufs=2, space="PSUM"))
        ops_pool = ctx.enter_context(tc.tile_pool(name="ops", bufs=2, space="PSUM"))

        ident8 = singles.tile([P, P], FP8)
        make_identity(nc, ident8)
        identb = singles.tile([P, P], BF16)
        make_identity(nc, identb)
        gamma_sb = singles.tile([P, 1], F32)
        nc.sync.dma_start(gamma_sb[:], gamma[:].to_broadcast((P, 1)))

        # PE HAM warmup (~3.5us of dummy matmuls while the first chunk loads):
        # transpose-mode work does not warm the clock gate, so without this
        # the first real matmuls run at 1.2GHz instead of 2.4.
        warm_src = singles.tile([P, 512], FP8)
        nc.vector.memset(warm_src[:], 0.0)
        warm_ps = ops_pool.tile([P, 512], F32, tag="ops", name="warm_ps")
        for w in range(16):
            nc.tensor.matmul(
                warm_ps[:], ident8[:], warm_src[:],
                start=(w == 0), stop=(w == 15),
            )

        states = {}

        def load_chunk(b, ch):
            st = states.setdefault(b, {"loaded": set()})
            if ch in st["loaded"]:
                return
            st["loaded"].add(ch)
            if "xb" not in st:
                st["xb"] = xb_pool.tile([P, J, N], BF16, tag="xb", name=f"xb{b}")
                st["xfc"] = xfc_pool.tile([P, J, N], FP8, tag="xfc", name=f"xfc{b}")
                st["xfT"] = xfT_pool.tile([P, KB, C], FP8, tag="xfT", name=f"xfT{b}")
                st["eps"] = [
                    eps_pool.tile([P, C], F32, tag="eps", name=f"eps{b}_{a}")
                    for a in range(J)
                ]
            xv = x[b].rearrange("(p j) n -> p j n", j=J)
            nsl = slice(ch * NW, (ch + 1) * NW)
            if b == 0 and ch == 0:
                # split the very first load per-j so the first casts (and with
                # them the first PE transposes) start earlier
                for j in range(J):
                    nc.gpsimd.dma_start(st["xb"][:, j, nsl], xv[:, j, nsl])
            else:
                nc.gpsimd.dma_start(st["xb"][:, :, nsl], xv[:, :, nsl])

        def cast_transpose(b, ch):
            """bf16->fp8 cast + PE transposes into xfT for chunk ch."""
            st = states[b]
            xb, xfc, xfT = st["xb"], st["xfc"], st["xfT"]
            nsl = slice(ch * NW, (ch + 1) * NW)
            # engine split is phase-aware: sample 0's casts run during the
            # load phase when DVE is idle; later samples' casts run while DVE
            # carries the previous sample's epilogue adds, so lean on ACT.
            dve_mod = 3 if b == 0 else 1
            for j in range(J):
                if (ch * J + j) % 4 < dve_mod:
                    nc.vector.tensor_copy(out=xfc[:, j, nsl], in_=xb[:, j, nsl])
                else:
                    nc.scalar.copy(out=xfc[:, j, nsl], in_=xb[:, j, nsl])
            # fp8 PE-transpose writes PSUM with element step 2 (16-bit write
            # packing): stage into a 2x-strided PSUM view, copy back strided.
            for kk in range(0, KPC, 2):
                kb = ch * KPC + kk
                tps = tps_pool.tile([P, 2, J, P * 2], FP8, tag="tps")
                wv = tps[:].rearrange("p u j (q t) -> p u j q t", t=2)[:, :, :, :, 0]
                for u in range(2):
                    for j in range(J):
                        nc.tensor.transpose(
                            wv[:, u, j, :],
                            xfc[:, j, (kb + u) * P:(kb + u + 1) * P],
                            ident8,
                        )
                dst = xfT[:, kb:kb + 2, :].rearrange("p u (j q) -> p u j q", q=P)
                if (ch * 2 + kk // 2) % 2 == 0:
                    nc.vector.tensor_copy(out=dst, in_=wv)
                else:
                    nc.scalar.copy(out=dst, in_=wv)

        def energy_accum(b, ch):
            """energy accumulation for chunk ch, fp8 DoubleRow over kb pairs."""
            st = states[b]
            xfT = st["xfT"]
            for a in range(J):
                e_ps = st["eps"][a]
                for kk in range(0, KPC, 2):
                    kb = ch * KPC + kk
                    nc.tensor.matmul(
                        e_ps[:],
                        xfT[:, kb:kb + 2, a * P:(a + 1) * P],
                        xfT[:, kb:kb + 2, :],
                        start=(kb == 0),
                        stop=(kb + 2 >= KB),
                        perf_mode=DR,
                    )

        def softmax(b):
            st = states[b]
            Pmat = pmat_pool.tile([P, J, C], FP8, tag="pmat")
            rS = small.tile([P, J], F32, tag="rS")
            for a in range(J):
                e_ps = st["eps"][a]
                m = small.tile([P, 1], F32, tag="m")
                nc.vector.tensor_reduce(
                    out=m[:], in_=e_ps[:], axis=mybir.AxisListType.X,
                    op=mybir.AluOpType.min,
                )
                S = small.tile([P, 1], F32, tag="S")
                nc.scalar.activation(
                    out=Pmat[:, a, :],
                    in_=e_ps[:],
                    func=mybir.ActivationFunctionType.Exp,
                    bias=m[:],
                    scale=-1.0,
                    accum_out=S[:],
                )
                nc.vector.reciprocal(out=rS[:, a:a + 1], in_=S[:])
            beta = small.tile([P, J], F32, tag="beta")
            nc.vector.tensor_tensor(
                out=beta[:],
                in0=rS[:],
                in1=gamma_sb[:].to_broadcast((P, J)),
                op=mybir.AluOpType.mult,
            )
            # pre-scale the numerator: row mi of Pmat *= beta[mi]; mm2 then
            # produces beta*(P@x) directly and the epilogue is just +x
            for a in range(J):
                nc.vector.tensor_scalar_mul(
                    out=Pmat[:, a, :], in0=Pmat[:, a, :], scalar1=beta[:, a:a + 1]
                )
            # PT tiles via PE transposes, grouped by source row-block ob so
            # each group starts as soon as exp/scale of that block lands
            PT = pt_pool.tile([P, J, C], FP8, tag="pt")
            for ob in range(J):
                tps = tps_pool.tile([P, J, P * 2], FP8, tag="tps")
                wv = tps[:].rearrange("p j (q t) -> p j q t", t=2)[:, :, :, 0]
                for cb in range(J):
                    nc.tensor.transpose(
                        wv[:, cb, :], Pmat[:, ob, cb * P:(cb + 1) * P], ident8
                    )
                dst = PT[:, :, ob * P:(ob + 1) * P]
                if ob % 2 == 0:
                    nc.vector.tensor_copy(out=dst, in_=wv)
                else:
                    nc.scalar.copy(out=dst, in_=wv)
            st["PT"] = PT

        def mm2_block(b, a):
            """output rows m in [a*128,(a+1)*128): o = beta*(P@x) + x, written
            in place over xb[:, a, :], then DMA'd out (8KB/partition rows)."""
            st = states[b]
            PT, xfc, xb = st["PT"], st["xfc"], st["xb"]
            for nh in range(NHC):
                nsl = slice(nh * NHW, (nh + 1) * NHW)
                o_ps = ops_pool.tile([P, NHW], F32, tag="ops")
                # 1 in 5 chunks: +x via identity matmul on PE with a pure ACT
                # copy epilogue; the rest: a DVE tensor add does the +x and
                # the PSUM->SBUF move in one op. Balances PE vs DVE vs ACT.
                use_ident = (a * NHC + nh) % 5 == 0
                for cb in (0, 2):
                    nc.tensor.matmul(
                        o_ps[:],
                        PT[:, cb:cb + 2, a * P:(a + 1) * P],
                        xfc[:, cb:cb + 2, nsl],
                        start=(cb == 0),
                        stop=(cb == 2 and not use_ident),
                        perf_mode=DR,
                    )
                if use_ident:
                    nc.tensor.matmul(
                        o_ps[:], identb[:], xb[:, a, nsl], start=False, stop=True
                    )
                    nc.scalar.copy(out=xb[:, a, nsl], in_=o_ps[:])
                else:
                    nc.vector.tensor_tensor(
                        out=xb[:, a, nsl],
                        in0=o_ps[:],
                        in1=xb[:, a, nsl],
                        op=mybir.AluOpType.add,
                    )
            yv = y[b].rearrange("(p j) n -> p j n", j=J)
            nc.sync.dma_start(yv[:, a, :], xb[:, a, :])

        # ---- software pipeline over the two samples ----
        for ch in range(NCH):
            load_chunk(0, ch)
        for ch in range(NCH):
            cast_transpose(0, ch)
            energy_accum(0, ch)
        for b in range(BPC):
            softmax(b)
            if b + 1 < BPC:
                for ch in range(NCH):
                    load_chunk(b + 1, ch)
                for ch in range(NCH):
                    # next sample's casts/transposes get priority over this
                    # sample's mm2: they gate the energy matmuls that keep PE
                    # fed right behind the loads
                    cast_transpose(b + 1, ch)
            for a in range(J):
                if b + 1 < BPC:
                    energy_accum(b + 1, a)
                mm2_block(b, a)

    nc.finalize()
    return nc


def kernel(x: np.ndarray, gamma: np.ndarray) -> np.ndarray:
    global LAST_EXEC_TIME_NS, LAST_TRACE, LAST_PROFILE_JSON
    from concourse.bass_utils import run_bass_kernel_spmd

    assert x.shape == (B, C, H, W), x.shape
    x = np.ascontiguousarray(x, dtype=np.float32)
    gamma = np.ascontiguousarray(gamma, dtype=np.float32).reshape(1)

    if "nc" not in _CACHE:
        _CACHE["nc"] = _build()
    nc = _CACHE["nc"]

    trace = os.environ.get("CAM_TRACE", "0") == "1" or bool(
        os.environ.get("BASS_TRACE")
    )
    kwargs = {}
    if trace:
        _ensure_ntff_hook()
        import tempfile

        tmpdir = tempfile.mkdtemp(prefix="cam_v2_trace_")
        try:
            os.unlink("/tmp/cam_v2_trace")
        except OSError:
            pass
        try:
            os.symlink(tmpdir, "/tmp/cam_v2_trace")
        except OSError:
            pass
        kwargs["tmpdir"] = tmpdir

    xs = x.reshape(N_CORES, BPC, C, N)
    in_maps = [{"x": xs[i], "gamma": gamma} for i in range(N_CORES)]
    res = run_bass_kernel_spmd(
        nc, in_maps, core_ids=list(range(N_CORES)), trace=trace, **kwargs
    )
    LAST_EXEC_TIME_NS = res.exec_time_ns
    LAST_TRACE = res.instructions_and_trace
    LAST_PROFILE_JSON = res.profile_json
    out = np.concatenate(
        [np.asarray(res.results[i]["y"]).astype(np.float32) for i in range(N_CORES)],
        axis=0,
    )
    return out.reshape(B, C, H, W)


# revision 3
# speedup vs baseline: 1.1962x; 1.0884x over previous
"""nn_CAM_Module kernel for 8 Trainium2 NeuronCores (Bass/Tile).

Contract: kernel(x: [16, 512, 64, 64] f32, gamma: [1] f32) -> full [16, 512,
64, 64] f32 output. Batch is sharded 2 samples/core across 8 cores, gamma
replicated (every op is a per-sample bmm, no cross-core communication).

Design (vs the chunked fp32 v1):
 - sigma channel layout: partition p holds channels {4p+j, j=0..3}, so DRAM
   reads are 4KB-contiguous per descriptor and y writes 8KB-contiguous.
 - loads are SWDGE DMA-casts fp32->bf16: x only ever lands in SBUF as bf16
   (the +x epilogue term and the fp8 matmul operands both derive from it),
   removing a full-sample engine cast. The bf16 rounding of the output is
   ~0.4% rel err, well inside the 2e-2 gate.
 - y is written as bf16 (halves HBM write traffic); the host upcasts to f32.
 - the softmax numerator is pre-scaled by beta = gamma/rowsum, so mm2
   produces beta*(P@x) directly. The +x is either added by the DVE epilogue
   op (which doubles as the PSUM->SBUF move) or, for a fraction of chunks,
   on the PE via an identity-bf16 matmul into the same PSUM accumulation
   with a pure copy epilogue on ACT - balancing PE/DVE/ACT load.
 - mm2 runs output-block-major: each y row-block (1MB) DMAs out as soon as
   its 8 PSUM chunks finish, in place over the bf16 x buffer.

Per-sample math (C=512, N=4096; m = j*128+p <-> c = 4p+j permutation; the
row softmax is permutation-invariant and inputs/outputs are permuted
consistently):
  energy = xf @ xf.T          (fp8 DoubleRow matmuls over PE-transposed tiles)
  P_ij   = beta_i * exp(min_j E_ij - E_ij),  beta = gamma / rowsum
  y      = P @ xf + x         (fp8 DR + epilogue add, written back as bf16)
"""

import os
from contextlib import ExitStack

import numpy as np

B, C, H, W = 16, 512, 64, 64
N = H * W
N_CORES = 8
BPC = B // N_CORES
P = 128
J = 4                 # channels per partition: c = 4p + j
NCH = 4               # load chunks per sample
NW = N // NCH         # 1024 spatial positions per chunk
KB = N // P           # 32 transpose blocks
KPC = NW // P         # 8 transpose blocks per chunk
NHC = 8               # mm2 psum chunks (512 wide)
NHW = N // NHC

LAST_EXEC_TIME_NS = None
LAST_TRACE = None
LAST_PROFILE_JSON = None
MM_DT_NAME = "fp8"    # informational; the kernel is fp8-DR + bf16 I/O
_CACHE = {}


def _ensure_ntff_hook():
    """Register the axon NTFF profile hook if the environment lacks
    antenv.axon_hooks (needed only when tracing; harmless otherwise)."""
    import sys
    import types

    try:
        from antenv.axon_hooks import get_axon_ntff_profile_hook  # noqa: F401
        return
    except ImportError:
        pass
    try:
        import antenv
        from trn_agent_boot.trn_boot import _ntff_profile_via_ctypes

        mod = types.ModuleType("antenv.axon_hooks")
        _hook = [None]
        mod.set_axon_ntff_profile_hook = lambda h: _hook.__setitem__(0, h)
        mod.get_axon_ntff_profile_hook = lambda: _hook[0]
        antenv.axon_hooks = mod
        sys.modules["antenv.axon_hooks"] = mod
        mod.set_axon_ntff_profile_hook(
            _ntff_profile_via_ctypes("/opt/axon/libaxon_pjrt.so")
        )
    except Exception:
        pass


def _build():
    import concourse.mybir as mybir
    import concourse.tile as tile
    from concourse import bacc
    from concourse.masks import make_identity

    F32 = mybir.dt.float32
    BF16 = mybir.dt.bfloat16
    FP8 = mybir.dt.float8e4
    DR = mybir.MatmulPerfMode.DoubleRow

    nc = bacc.Bacc(None, target_bir_lowering=False, debug=False)
    x = nc.dram_tensor("x", [BPC, C, N], F32, kind="ExternalInput")
    gamma = nc.dram_tensor("gamma", [1], F32, kind="ExternalInput")
    y = nc.dram_tensor("y", [BPC, C, N], BF16, kind="ExternalOutput")

    with ExitStack() as ctx:
        tc = ctx.enter_context(tile.TileContext(nc))
        singles = ctx.enter_context(tc.tile_pool(name="singles", bufs=1))
        xb_pool = ctx.enter_context(tc.tile_pool(name="xb", bufs=2))
        xfc_pool = ctx.enter_context(tc.tile_pool(name="xfc", bufs=2))
        xfT_pool = ctx.enter_context(tc.tile_pool(name="xfT", bufs=2))
        pmat_pool = ctx.enter_context(tc.tile_pool(name="pmat", bufs=2))
        pt_pool = ctx.enter_context(tc.tile_pool(name="pt", bufs=2))
        small = ctx.enter_context(tc.tile_pool(name="small", bufs=16))
        eps_pool = ctx.enter_context(tc.tile_pool(name="eps", bufs=4, space="PSUM"))
        tps_pool = ctx.enter_context(tc.tile_pool(name="tps", bufs=2, space="PSUM"))
        ops_pool = ctx.enter_context(tc.tile_pool(name="ops", bufs=2, space="PSUM"))

        ident8 = singles.tile([P, P], FP8)
        make_identity(nc, ident8)
        identb = singles.tile([P, P], BF16)
        make_identity(nc, identb)
        gamma_sb = singles.tile([P, 1], F32)
        nc.sync.dma_start(gamma_sb[:], gamma[:].to_broadcast((P, 1)))

        # PE HAM warmup (~3.5us of dummy matmuls while the first chunk loads):
        # transpose-mode work does not warm the clock gate, so without this
        # the first real matmuls run at 1.2GHz instead of 2.4.
        warm_src = singles.tile([P, 512], FP8)
        nc.vector.memset(warm_src[:], 0.0)
        warm_ps = ops_pool.tile([P, 512], F32, tag="ops", name="warm_ps")
        for w in range(16):
            nc.tensor.matmul(
                warm_ps[:], ident8[:], warm_src[:],
                start=(w == 0), stop=(w == 15),
            )

        states = {}

        def load_chunk(b, ch):
            st = states.setdefault(b, {"loaded": set()})
            if ch in st["loaded"]:
                return
            st["loaded"].add(ch)
            if "xb" not in st:
                st["xb"] = xb_pool.tile([P, J, N], BF16, tag="xb", name=f"xb{b}")
                st["xfc"] = xfc_pool.tile([P, J, N], FP8, tag="xfc", name=f"xfc{b}")
                st["xfT"] = xfT_pool.tile([P, KB, C], FP8, tag="xfT", name=f"xfT{b}")
                st["eps"] = [
                    eps_pool.tile([P, C], F32, tag="eps", name=f"eps{b}_{a}")
                    for a in range(J)
                ]
            xv = x[b].rearrange("(p j) n -> p j n", j=J)
            nsl = slice(ch * NW, (ch + 1) * NW)
            if b == 0 and ch == 0:
                # split the very first load per-j so the first casts (and with
                # them the first PE transposes) start earlier
                for j in range(J):
                    nc.gpsimd.dma_start(st["xb"][:, j, nsl], xv[:, j, nsl])
            else:
                nc.gpsimd.dma_start(st["xb"][:, :, nsl], xv[:, :, nsl])

        def cast_transpose(b, ch):
            """bf16->fp8 cast + PE transposes into xfT for chunk ch."""
            st = states[b]
            xb, xfc, xfT = st["xb"], st["xfc"], st["xfT"]
            nsl = slice(ch * NW, (ch + 1) * NW)
            # engine split is phase-aware: sample 0's casts run during the
            # load phase when DVE is idle; later samples' casts run while DVE
            # carries the previous sample's epilogue adds, so lean on ACT.
            dve_mod = 1
            for j in range(J):
                if (ch * J + j) % 4 < dve_mod:
                    nc.vector.tensor_copy(out=xfc[:, j, nsl], in_=xb[:, j, nsl])
                else:
                    nc.scalar.copy(out=xfc[:, j, nsl], in_=xb[:, j, nsl])
            # fp8 PE-transpose writes PSUM with element step 2 (16-bit write
            # packing): stage into a 2x-strided PSUM view, copy back strided.
            for kk in range(0, KPC, 2):
                kb = ch * KPC + kk
                tps = tps_pool.tile([P, 2, J, P * 2], FP8, tag="tps")
                wv = tps[:].rearrange("p u j (q t) -> p u j q t", t=2)[:, :, :, :, 0]
                for u in range(2):
                    for j in range(J):
                        nc.tensor.transpose(
                            wv[:, u, j, :],
                            xfc[:, j, (kb + u) * P:(kb + u + 1) * P],
                            ident8,
                        )
                dst = xfT[:, kb:kb + 2, :].rearrange("p u (j q) -> p u j q", q=P)
                if (ch * 2 + kk // 2) % 2 == 0:
                    nc.vector.tensor_copy(out=dst, in_=wv)
                else:
                    nc.scalar.copy(out=dst, in_=wv)

        def energy_accum(b, ch):
            """energy accumulation for chunk ch, fp8 DoubleRow over kb pairs."""
            st = states[b]
            xfT = st["xfT"]
            for a in range(J):
                e_ps = st["eps"][a]
                for kk in range(0, KPC, 2):
                    kb = ch * KPC + kk
                    nc.tensor.matmul(
                        e_ps[:],
                        xfT[:, kb:kb + 2, a * P:(a + 1) * P],
                        xfT[:, kb:kb + 2, :],
                        start=(kb == 0),
                        stop=(kb + 2 >= KB),
                        perf_mode=DR,
                    )

        def softmax(b):
            st = states[b]
            Pmat = pmat_pool.tile([P, J, C], FP8, tag="pmat")
            rS = small.tile([P, J], F32, tag="rS")
            for a in range(J):
                e_ps = st["eps"][a]
                m = small.tile([P, 1], F32, tag="m")
                nc.vector.tensor_reduce(
                    out=m[:], in_=e_ps[:], axis=mybir.AxisListType.X,
                    op=mybir.AluOpType.min,
                )
                S = small.tile([P, 1], F32, tag="S")
                nc.scalar.activation(
                    out=Pmat[:, a, :],
                    in_=e_ps[:],
                    func=mybir.ActivationFunctionType.Exp,
                    bias=m[:],
                    scale=-1.0,
                    accum_out=S[:],
                )
                nc.vector.reciprocal(out=rS[:, a:a + 1], in_=S[:])
            beta = small.tile([P, J], F32, tag="beta")
            nc.vector.tensor_tensor(
                out=beta[:],
                in0=rS[:],
                in1=gamma_sb[:].to_broadcast((P, J)),
                op=mybir.AluOpType.mult,
            )
            # pre-scale the numerator: row mi of Pmat *= beta[mi]; mm2 then
            # produces beta*(P@x) directly and the epilogue is just +x
            for a in range(J):
                nc.vector.tensor_scalar_mul(
                    out=Pmat[:, a, :], in0=Pmat[:, a, :], scalar1=beta[:, a:a + 1]
                )
            # PT tiles via PE transposes, grouped by source row-block ob so
            # each group starts as soon as exp/scale of that block lands
            PT = pt_pool.tile([P, J, C], FP8, tag="pt")
            for ob in range(J):
                tps = tps_pool.tile([P, J, P * 2], FP8, tag="tps")
                wv = tps[:].rearrange("p j (q t) -> p j q t", t=2)[:, :, :, 0]
                for cb in range(J):
                    nc.tensor.transpose(
                        wv[:, cb, :], Pmat[:, ob, cb * P:(cb + 1) * P], ident8
                    )
                dst = PT[:, :, ob * P:(ob + 1) * P]
                if ob % 2 == 0:
                    nc.vector.tensor_copy(out=dst, in_=wv)
                else:
                    nc.scalar.copy(out=dst, in_=wv)
            st["PT"] = PT

        def mm2_block(b, a):
            """output rows m in [a*128,(a+1)*128): o = beta*(P@x) + x, written
            in place over xb[:, a, :], then DMA'd out (8KB/partition rows)."""
            st = states[b]
            PT, xfc, xb = st["PT"], st["xfc"], st["xb"]
            for nh in range(NHC):
                nsl = slice(nh * NHW, (nh + 1) * NHW)
                o_ps = ops_pool.tile([P, NHW], F32, tag="ops")
                # 1 in 5 chunks: +x via identity matmul on PE with a pure ACT
                # copy epilogue; the rest: a DVE tensor add does the +x and
                # the PSUM->SBUF move in one op. Balances PE vs DVE vs ACT.
                use_ident = (a * NHC + nh) % 5 == 0
                for cb in (0, 2):
                    nc.tensor.matmul(
                        o_ps[:],
                        PT[:, cb:cb + 2, a * P:(a + 1) * P],
                        xfc[:, cb:cb + 2, nsl],
                        start=(cb == 0),
                        stop=(cb == 2 and not use_ident),
                        perf_mode=DR,
                    )
                if use_ident:
                    nc.tensor.matmul(
                        o_ps[:], identb[:], xb[:, a, nsl], start=False, stop=True
                    )
                    nc.scalar.copy(out=xb[:, a, nsl], in_=o_ps[:])
                else:
                    nc.vector.tensor_tensor(
                        out=xb[:, a, nsl],
                        in0=o_ps[:],
                        in1=xb[:, a, nsl],
                        op=mybir.AluOpType.add,
                    )
            yv = y[b].rearrange("(p j) n -> p j n", j=J)
            nc.sync.dma_start(yv[:, a, :], xb[:, a, :])

        # ---- software pipeline over the two samples ----
        for ch in range(NCH):
            load_chunk(0, ch)
        for ch in range(NCH):
            cast_transpose(0, ch)
            energy_accum(0, ch)
        for b in range(BPC):
            softmax(b)
            if b + 1 < BPC:
                for ch in range(NCH):
                    load_chunk(b + 1, ch)
                for ch in range(NCH):
                    # next sample's casts/transposes get priority over this
                    # sample's mm2: they gate the energy matmuls that keep PE
                    # fed right behind the loads
                    cast_transpose(b + 1, ch)
            for a in range(J):
                if b + 1 < BPC:
                    energy_accum(b + 1, a)
                mm2_block(b, a)

    nc.finalize()
    return nc


def kernel(x: np.ndarray, gamma: np.ndarray) -> np.ndarray:
    global LAST_EXEC_TIME_NS, LAST_TRACE, LAST_PROFILE_JSON
    from concourse.bass_utils import run_bass_kernel_spmd

    assert x.shape == (B, C, H, W), x.shape
    x = np.ascontiguousarray(x, dtype=np.float32)
    gamma = np.ascontiguousarray(gamma, dtype=np.float32).reshape(1)

    if "nc" not in _CACHE:
        _CACHE["nc"] = _build()
    nc = _CACHE["nc"]

    trace = os.environ.get("CAM_TRACE", "0") == "1" or bool(
        os.environ.get("BASS_TRACE")
    )
    kwargs = {}
    if trace:
        _ensure_ntff_hook()
        import tempfile

        tmpdir = tempfile.mkdtemp(prefix="cam_v2_trace_")
        try:
            os.unlink("/tmp/cam_v2_trace")
        except OSError:
            pass
        try:
            os.symlink(tmpdir, "/tmp/cam_v2_trace")
        except OSError:
            pass
        kwargs["tmpdir"] = tmpdir

    xs = x.reshape(N_CORES, BPC, C, N)
    in_maps = [{"x": xs[i], "gamma": gamma} for i in range(N_CORES)]
    res = run_bass_kernel_spmd(
        nc, in_maps, core_ids=list(range(N_CORES)), trace=trace, **kwargs
    )
    LAST_EXEC_TIME_NS = res.exec_time_ns
    LAST_TRACE = res.instructions_and_trace
    LAST_PROFILE_JSON = res.profile_json
    out = np.concatenate(
        [np.asarray(res.results[i]["y"]).astype(np.float32) for i in range(N_CORES)],
        axis=0,
    )
    return out.reshape(B, C, H, W)


# revision 8
# speedup vs baseline: 1.2735x; 1.0646x over previous
"""nn_CAM_Module kernel for 8 Trainium2 NeuronCores (Bass/Tile).

Contract: kernel(**inputs) takes the FULL inputs (x: [16, 512, 64, 64] fp32,
gamma: [1] fp32) and returns the FULL output, sharding batch B=16 across the
8 cores (2 samples per core, gamma replicated) — per the data-parallel
sharding: every op is a per-sample bmm, no cross-core communication.

Per-sample computation (C=512 channels, N=H*W=4096):
  energy = xf @ xf.T                          (C,C), contraction over N on PE
  m_i    = min_j energy[i,j]                  (softmax(max-e) == softmax(m-e))
  P_ij   = exp(m_i - energy_ij), S_i = sum_j  (ACT, fused row-sum)
  out    = diag(1/S) @ (P @ xf)               (PE; P^T tiles via PE transpose)
  y      = gamma * out + x                    (fused DVE mult-add)

Layouts per core (P=128 partitions):
  xf   [128, 4, 4096] fp32   channel blocks on partitions (DMA from DRAM)
  xfc  [128, 4, 4096] mm_dt  low-precision cast (matmul operand)
  xfT  [128, 32, 512] mm_dt  spatial chunks on partitions (PE transposes)
  Pmat [128, 4, 512]  mm_dt  attention numerator, rows i
  PT   [128, 4, 512]  mm_dt  P^T tiles (PE transposes), matmul2 stationary
"""

import os
from contextlib import ExitStack

import numpy as np

B, C, H, W = 16, 512, 64, 64
N = H * W
N_CORES = 8
BPC = B // N_CORES
P = 128

MM_DT_NAME = os.environ.get("CAM_MM_DT", "fp8")

LAST_EXEC_TIME_NS = None
LAST_TRACE = None
LAST_PROFILE_JSON = None
_CACHE = {}


def _build(mm_dt_name):
    import concourse.mybir as mybir
    import concourse.tile as tile
    from concourse import bacc
    from concourse.masks import make_identity

    F32 = mybir.dt.float32
    BF16 = mybir.dt.bfloat16
    mm_dt = {
        "bf16": mybir.dt.bfloat16,
        "fp8": mybir.dt.float8e4,
        "f32": F32,
    }[mm_dt_name]
    DR = mm_dt in (mybir.dt.float8e4, mybir.dt.float8e5)

    CB = C // P          # 4 channel blocks
    KB = N // P          # 32 spatial chunks
    NCH_SZ = 512
    NCH = N // NCH_SZ    # 8 output column chunks

    nc = bacc.Bacc(None, target_bir_lowering=False, debug=False)
    x = nc.dram_tensor("x", [BPC, C, N], F32, kind="ExternalInput")
    gamma = nc.dram_tensor("gamma", [1], F32, kind="ExternalInput")
    # y is written bf16 (halves HBM write traffic; host upcasts). With the
    # rel-err gate at 2e-2, bf16 rounding of the output (~0.4%) is safe.
    y = nc.dram_tensor("y", [BPC, C, N], BF16, kind="ExternalOutput")

    with ExitStack() as ctx:
        tc = ctx.enter_context(tile.TileContext(nc))
        singles = ctx.enter_context(tc.tile_pool(name="singles", bufs=1))
        xf_pool = ctx.enter_context(tc.tile_pool(name="xf", bufs=12))
        xfc_pool = ctx.enter_context(tc.tile_pool(name="xfc", bufs=12))
        xfT_pool = ctx.enter_context(tc.tile_pool(name="xfT", bufs=2))
        pmat_pool = ctx.enter_context(tc.tile_pool(name="pmat", bufs=2))
        pt_pool = ctx.enter_context(tc.tile_pool(name="pt", bufs=2))
        small = ctx.enter_context(tc.tile_pool(name="small", bufs=16))
        yt_pool = ctx.enter_context(tc.tile_pool(name="yt", bufs=3))
        eps_pool = ctx.enter_context(tc.tile_pool(name="eps", bufs=4, space="PSUM"))
        tps_pool = ctx.enter_context(tc.tile_pool(name="tps", bufs=2, space="PSUM"))
        ops_pool = ctx.enter_context(tc.tile_pool(name="ops", bufs=2, space="PSUM"))

        ident = singles.tile([P, P], mm_dt)
        make_identity(nc, ident)
        gamma_sb = singles.tile([P, 1], F32)
        nc.sync.dma_start(gamma_sb[:], gamma[:].to_broadcast((P, 1)))

        # ~3.5us of dummy matmuls while the first chunk loads: warms the
        # PE HAM clock-gate (transpose-mode work doesn't), so the first
        # real transposes run at 2.4GHz instead of 1.2.
        warm_src = singles.tile([P, 512], mm_dt)
        nc.vector.memset(warm_src[:], 0.0)
        warm_ps = ops_pool.tile([P, NCH_SZ], F32, tag="ops", name="warm_ps")
        for w in range(16):
            nc.tensor.matmul(
                warm_ps[:], ident[:], warm_src[:],
                start=(w == 0), stop=(w == 15),
            )

        # fp8 PE-transpose writes PSUM with element step 2 (16-bit write
        # packing): stage into a 2x-strided PSUM view, copy back strided.
        TW = 2 if DR else 1
        KPC = NCH_SZ // P  # transposes-k per n-chunk

        def tps_views(tps):
            if TW == 1:
                return tps, tps
            v = tps[:].rearrange("p cb (n t) -> p cb n t", t=TW)[:, :, :, 0]
            return v, v

        # ---- software pipeline over samples ----
        # prefetch_chunk(b, ch): load 1MB n-chunk, cast, PE-transpose into
        #   xfT, and accumulate this chunk's k-pairs into the energy PSUMs.
        # softmax(b): row-min + exp(+rowsum) + beta + P^T tiles.
        # mm2_chunk(b, nh): attention matmul + fused epilogue + y write.
        # Emission interleaves sample b's mm2 chunks with sample b+1's
        # prefetch chunks so neither PE nor DMA drains between samples.
        states = {}

        def load_chunk(b, ch):
            """DMA-only part: issue the 1MB chunk load (sync queue). Safe to
            hoist ahead of the previous sample's softmax/mm2 emission — it
            adds no PE/DVE/ACT work there, just keeps the DMA engines fed."""
            st = states.setdefault(b, {"xf": [], "xfc": []})
            if len(st["xf"]) > ch:
                return
            xv = x[b].rearrange("(cb p) n -> p cb n", p=P)
            nsl = slice(ch * NCH_SZ, (ch + 1) * NCH_SZ)
            xfch = xf_pool.tile([P, CB, NCH_SZ], F32, tag="xf", name=f"xf{b}_{ch}")
            if b == 0 and ch == 0:
                # split the very first load per-cb so the first cast (and
                # with it the first PE transpose) starts ~3us earlier
                for cb in range(CB):
                    nc.sync.dma_start(xfch[:, cb, :], xv[:, cb, nsl])
            else:
                nc.sync.dma_start(xfch[:], xv[:, :, nsl])
            st["xf"].append(xfch)

        def prefetch_chunk(b, ch):
            load_chunk(b, ch)
            st = states[b]
            if "xfT" not in st:
                st["xfT"] = xfT_pool.tile([P, KB, C], mm_dt, tag="xfT", name=f"xfT{b}")
                st["eps"] = [
                    eps_pool.tile([P, C], F32, tag="eps", name=f"eps{b}_{i}")
                    for i in range(CB)
                ]
            xfch = st["xf"][ch]
            xfcch = xfc_pool.tile([P, CB, NCH_SZ], mm_dt, tag="xfc")
            # fine-grained per-cb casts so the first transposes start
            # right after the first sub-cast, split across DVE/ACT
            for cb in range(CB):
                # 3:1 toward ACT: DVE is the busier engine (epilogue+copies)
                if (ch * CB + cb) % 4 == 0:
                    nc.vector.tensor_copy(out=xfcch[:, cb, :], in_=xfch[:, cb, :])
                else:
                    nc.scalar.copy(out=xfcch[:, cb, :], in_=xfch[:, cb, :])
            st["xfc"].append(xfcch)
            xfT = st["xfT"]
            # two k-groups share one PSUM bank: 8 transposes, one copy
            for kk in range(0, KPC, 2):
                k = ch * KPC + kk
                tps = tps_pool.tile([P, 2, CB, P * TW], mm_dt, tag="tps")
                if TW == 1:
                    wv = tps[:]
                else:
                    wv = tps[:].rearrange("p u cb (n t) -> p u cb n t", t=TW)[
                        :, :, :, :, 0
                    ]
                for u in range(2):
                    for cb in range(CB):
                        nc.tensor.transpose(
                            wv[:, u, cb, :],
                            xfcch[:, cb, (kk + u) * P : (kk + u + 1) * P],
                            ident,
                        )
                dst = xfT[:, k : k + 2, :].rearrange("p u (cb n) -> p u cb n", n=P)
                # xfT copies mostly on ACT to unload DVE
                if (ch * 2 + kk // 2) % 4 == 0:
                    nc.vector.tensor_copy(out=dst, in_=wv)
                else:
                    nc.scalar.copy(out=dst, in_=wv)
            # energy accumulation for this chunk's k-pairs
            for cb in range(CB):
                e_ps = st["eps"][cb]
                if DR:
                    for kk in range(0, KPC, 2):
                        k = ch * KPC + kk
                        nc.tensor.matmul(
                            e_ps[:],
                            xfT[:, k : k + 2, cb * P : (cb + 1) * P],
                            xfT[:, k : k + 2, :],
                            start=(k == 0),
                            stop=(k + 2 >= KB),
                            perf_mode=mybir.MatmulPerfMode.DoubleRow,
                        )
                else:
                    for kk in range(KPC):
                        k = ch * KPC + kk
                        nc.tensor.matmul(
                            e_ps[:],
                            xfT[:, k, cb * P : (cb + 1) * P],
                            xfT[:, k, :],
                            start=(k == 0),
                            stop=(k == KB - 1),
                        )

        def softmax(b):
            st = states[b]
            Pmat = pmat_pool.tile([P, CB, C], mm_dt, tag="pmat")
            rS = small.tile([P, CB], F32, tag="rS")
            for cb in range(CB):
                e_ps = st["eps"][cb]
                m = small.tile([P, 1], F32, tag="m")
                nc.vector.tensor_reduce(
                    out=m[:], in_=e_ps[:], axis=mybir.AxisListType.X,
                    op=mybir.AluOpType.min,
                )
                S = small.tile([P, 1], F32, tag="S")
                nc.scalar.activation(
                    out=Pmat[:, cb, :],
                    in_=e_ps[:],
                    func=mybir.ActivationFunctionType.Exp,
                    bias=m[:],
                    scale=-1.0,
                    accum_out=S[:],
                )
                nc.vector.reciprocal(out=rS[:, cb : cb + 1], in_=S[:])

            beta = small.tile([P, CB], F32, tag="beta")
            nc.vector.tensor_tensor(
                out=beta[:],
                in0=rS[:],
                in1=gamma_sb[:].to_broadcast((P, CB)),
                op=mybir.AluOpType.mult,
            )
            st["beta"] = beta

            # PT transposes grouped by source row-block ob so each group can
            # start as soon as exp(ob) lands (no wait for all four exps).
            PT = pt_pool.tile([P, CB, C], mm_dt, tag="pt")
            for ob in range(CB):
                tps = tps_pool.tile([P, CB, P * TW], mm_dt, tag="tps")
                wv, rv = tps_views(tps)
                for cb in range(CB):
                    nc.tensor.transpose(
                        wv[:, cb, :], Pmat[:, ob, cb * P : (cb + 1) * P], ident
                    )
                dst = PT[:, :, ob * P : (ob + 1) * P]
                if ob % 2 == 0:
                    nc.vector.tensor_copy(out=dst, in_=rv)
                else:
                    nc.scalar.copy(out=dst, in_=rv)
            st["PT"] = PT

        def mm2_chunk(b, nh):
            st = states[b]
            PT, beta = st["PT"], st["beta"]
            yv = y[b].rearrange("(ob p) n -> p ob n", p=P)
            nsl = slice(nh * NCH_SZ, (nh + 1) * NCH_SZ)
            yt = yt_pool.tile([P, CB, NCH_SZ], BF16, tag="yt")
            for ob in range(CB):
                o_ps = ops_pool.tile([P, NCH_SZ], F32, tag="ops")
                if DR:
                    for cb in range(0, CB, 2):
                        nc.tensor.matmul(
                            o_ps[:],
                            PT[:, cb : cb + 2, ob * P : (ob + 1) * P],
                            st["xfc"][nh][:, cb : cb + 2, :],
                            start=(cb == 0),
                            stop=(cb + 2 >= CB),
                            perf_mode=mybir.MatmulPerfMode.DoubleRow,
                        )
                else:
                    for cb in range(CB):
                        nc.tensor.matmul(
                            o_ps[:],
                            PT[:, cb, ob * P : (ob + 1) * P],
                            st["xfc"][nh][:, cb, :],
                            start=(cb == 0),
                            stop=(cb == CB - 1),
                        )
                nc.vector.scalar_tensor_tensor(
                    out=yt[:, ob, :],
                    in0=o_ps[:],
                    scalar=beta[:, ob : ob + 1],
                    in1=st["xf"][nh][:, ob, :],
                    op0=mybir.AluOpType.mult,
                    op1=mybir.AluOpType.add,
                )
            # SWDGE so writes don't block the next sample's loads in the
            # HWDGE FIFO (gpsimd engine is otherwise idle)
            nc.gpsimd.dma_start(yv[:, :, nsl], yt[:])

        for ch in range(NCH):
            prefetch_chunk(0, ch)
        for b in range(BPC):
            if b + 1 < BPC:
                # hoist the next sample's first loads (DMA only) so they
                # queue right behind this sample's loads on the sync FIFO
                for ch in range(min(4, NCH)):
                    load_chunk(b + 1, ch)
            softmax(b)
            for nh in range(NCH):
                mm2_chunk(b, nh)
                if b + 1 < BPC:
                    prefetch_chunk(b + 1, nh)

    nc.finalize()
    return nc


def kernel(x: np.ndarray, gamma: np.ndarray) -> np.ndarray:
    global LAST_EXEC_TIME_NS, LAST_TRACE, LAST_PROFILE_JSON
    from concourse.bass_utils import run_bass_kernel_spmd

    assert x.shape == (B, C, H, W), x.shape
    x = np.ascontiguousarray(x, dtype=np.float32)
    gamma = np.ascontiguousarray(gamma, dtype=np.float32).reshape(1)

    name = MM_DT_NAME
    if name not in _CACHE:
        _CACHE[name] = _build(name)
    nc = _CACHE[name]

    xs = x.reshape(N_CORES, BPC, C, N)
    in_maps = [{"x": xs[i], "gamma": gamma} for i in range(N_CORES)]
    trace = os.environ.get("CAM_TRACE", "0") == "1"
    kwargs = {}
    if trace:
        import tempfile

        tmpdir = tempfile.mkdtemp(prefix=f"cam_trace_{name}_")
        try:
            os.unlink(f"/tmp/cam_trace_{name}")
        except OSError:
            pass
        os.symlink(tmpdir, f"/tmp/cam_trace_{name}")
        kwargs["tmpdir"] = tmpdir
    res = run_bass_kernel_spmd(
        nc, in_maps, core_ids=list(range(N_CORES)), trace=trace, **kwargs
    )
    LAST_EXEC_TIME_NS = res.exec_time_ns
    LAST_TRACE = res.instructions_and_trace
    LAST_PROFILE_JSON = res.profile_json
    out = np.concatenate(
        [np.asarray(res.results[i]["y"]).astype(np.float32) for i in range(N_CORES)],
        axis=0,
    )
    return out.reshape(B, C, H, W)



# revision 10
# speedup vs baseline: 1.2846x; 1.0088x over previous
"""nn_CAM_Module kernel for 8 Trainium2 NeuronCores (Bass/Tile).

Contract: kernel(**inputs) takes the FULL inputs (x: [16, 512, 64, 64] fp32,
gamma: [1] fp32) and returns the FULL output, sharding batch B=16 across the
8 cores (2 samples per core, gamma replicated) — per the data-parallel
sharding: every op is a per-sample bmm, no cross-core communication.

Per-sample computation (C=512 channels, N=H*W=4096):
  energy = xf @ xf.T                          (C,C), contraction over N on PE
  m_i    = min_j energy[i,j]                  (softmax(max-e) == softmax(m-e))
  P_ij   = exp(m_i - energy_ij), S_i = sum_j  (ACT, fused row-sum)
  out    = diag(1/S) @ (P @ xf)               (PE; P^T tiles via PE transpose)
  y      = gamma * out + x                    (fused DVE mult-add)

Layouts per core (P=128 partitions):
  xf   [128, 4, 4096] fp32   channel blocks on partitions (DMA from DRAM)
  xfc  [128, 4, 4096] mm_dt  low-precision cast (matmul operand)
  xfT  [128, 32, 512] mm_dt  spatial chunks on partitions (PE transposes)
  Pmat [128, 4, 512]  mm_dt  attention numerator, rows i
  PT   [128, 4, 512]  mm_dt  P^T tiles (PE transposes), matmul2 stationary
"""

import os
from contextlib import ExitStack

import numpy as np

B, C, H, W = 16, 512, 64, 64
N = H * W
N_CORES = 8
BPC = B // N_CORES
P = 128

MM_DT_NAME = os.environ.get("CAM_MM_DT", "fp8")

LAST_EXEC_TIME_NS = None
LAST_TRACE = None
LAST_PROFILE_JSON = None
_CACHE = {}


def _build(mm_dt_name):
    import concourse.mybir as mybir
    import concourse.tile as tile
    from concourse import bacc
    from concourse.masks import make_identity

    F32 = mybir.dt.float32
    BF16 = mybir.dt.bfloat16
    mm_dt = {
        "bf16": mybir.dt.bfloat16,
        "fp8": mybir.dt.float8e4,
        "f32": F32,
    }[mm_dt_name]
    DR = mm_dt in (mybir.dt.float8e4, mybir.dt.float8e5)

    CB = C // P          # 4 channel blocks
    KB = N // P          # 32 spatial chunks
    NCH_SZ = 512
    NCH = N // NCH_SZ    # 8 output column chunks

    nc = bacc.Bacc(None, target_bir_lowering=False, debug=False)
    x = nc.dram_tensor("x", [BPC, C, N], F32, kind="ExternalInput")
    gamma = nc.dram_tensor("gamma", [1], F32, kind="ExternalInput")
    # y is written bf16 (halves HBM write traffic; host upcasts). With the
    # rel-err gate at 2e-2, bf16 rounding of the output (~0.4%) is safe.
    y = nc.dram_tensor("y", [BPC, C, N], BF16, kind="ExternalOutput")

    with ExitStack() as ctx:
        tc = ctx.enter_context(tile.TileContext(nc))
        singles = ctx.enter_context(tc.tile_pool(name="singles", bufs=1))
        xf_pool = ctx.enter_context(tc.tile_pool(name="xf", bufs=12))
        xfc_pool = ctx.enter_context(tc.tile_pool(name="xfc", bufs=12))
        xfT_pool = ctx.enter_context(tc.tile_pool(name="xfT", bufs=2))
        pmat_pool = ctx.enter_context(tc.tile_pool(name="pmat", bufs=2))
        pt_pool = ctx.enter_context(tc.tile_pool(name="pt", bufs=2))
        small = ctx.enter_context(tc.tile_pool(name="small", bufs=16))
        yt_pool = ctx.enter_context(tc.tile_pool(name="yt", bufs=3))
        eps_pool = ctx.enter_context(tc.tile_pool(name="eps", bufs=4, space="PSUM"))
        tps_pool = ctx.enter_context(tc.tile_pool(name="tps", bufs=2, space="PSUM"))
        ops_pool = ctx.enter_context(tc.tile_pool(name="ops", bufs=2, space="PSUM"))

        ident = singles.tile([P, P], mm_dt)
        make_identity(nc, ident)
        gamma_sb = singles.tile([P, 1], F32)
        nc.sync.dma_start(gamma_sb[:], gamma[:].to_broadcast((P, 1)))

        # ~3.5us of dummy matmuls while the first chunk loads: warms the
        # PE HAM clock-gate (transpose-mode work doesn't), so the first
        # real transposes run at 2.4GHz instead of 1.2.
        warm_src = singles.tile([P, 512], mm_dt)
        nc.vector.memset(warm_src[:], 0.0)
        warm_ps = ops_pool.tile([P, NCH_SZ], F32, tag="ops", name="warm_ps")
        for w in range(16):
            nc.tensor.matmul(
                warm_ps[:], ident[:], warm_src[:],
                start=(w == 0), stop=(w == 15),
            )

        # fp8 PE-transpose writes PSUM with element step 2 (16-bit write
        # packing): stage into a 2x-strided PSUM view, copy back strided.
        TW = 2 if DR else 1
        KPC = NCH_SZ // P  # transposes-k per n-chunk

        def tps_views(tps):
            if TW == 1:
                return tps, tps
            v = tps[:].rearrange("p cb (n t) -> p cb n t", t=TW)[:, :, :, 0]
            return v, v

        # ---- software pipeline over samples ----
        # prefetch_chunk(b, ch): load 1MB n-chunk, cast, PE-transpose into
        #   xfT, and accumulate this chunk's k-pairs into the energy PSUMs.
        # softmax(b): row-min + exp(+rowsum) + beta + P^T tiles.
        # mm2_chunk(b, nh): attention matmul + fused epilogue + y write.
        # Emission interleaves sample b's mm2 chunks with sample b+1's
        # prefetch chunks so neither PE nor DMA drains between samples.
        states = {}

        def load_chunk(b, ch):
            """DMA-only part: issue the 1MB chunk load (sync queue). Safe to
            hoist ahead of the previous sample's softmax/mm2 emission — it
            adds no PE/DVE/ACT work there, just keeps the DMA engines fed."""
            st = states.setdefault(b, {"xf": [], "xfc": []})
            if len(st["xf"]) > ch:
                return
            xv = x[b].rearrange("(cb p) n -> p cb n", p=P)
            nsl = slice(ch * NCH_SZ, (ch + 1) * NCH_SZ)
            xfch = xf_pool.tile([P, CB, NCH_SZ], F32, tag="xf", name=f"xf{b}_{ch}")
            if b == 0 and ch == 0:
                # split the very first load per-cb so the first cast (and
                # with it the first PE transpose) starts ~3us earlier
                for cb in range(CB):
                    nc.sync.dma_start(xfch[:, cb, :], xv[:, cb, nsl])
            else:
                nc.sync.dma_start(xfch[:], xv[:, :, nsl])
            st["xf"].append(xfch)

        def prefetch_chunk(b, ch):
            load_chunk(b, ch)
            st = states[b]
            if "xfT" not in st:
                st["xfT"] = xfT_pool.tile([P, KB, C], mm_dt, tag="xfT", name=f"xfT{b}")
                st["eps"] = [
                    eps_pool.tile([P, C], F32, tag="eps", name=f"eps{b}_{i}")
                    for i in range(CB)
                ]
            xfch = st["xf"][ch]
            xfcch = xfc_pool.tile([P, CB, NCH_SZ], mm_dt, tag="xfc")
            # fine-grained per-cb casts so the first transposes start
            # right after the first sub-cast, split across DVE/ACT
            for cb in range(CB):
                # sample 0's casts run in the load phase where DVE is idle
                # (no epilogue yet): split 2:2. Later samples: 3:1 toward ACT
                # since DVE carries the epilogue then.
                dve_share = 2 if b == 0 else 1
                if (ch * CB + cb) % 4 < dve_share:
                    nc.vector.tensor_copy(out=xfcch[:, cb, :], in_=xfch[:, cb, :])
                else:
                    nc.scalar.copy(out=xfcch[:, cb, :], in_=xfch[:, cb, :])
            st["xfc"].append(xfcch)
            xfT = st["xfT"]
            # two k-groups share one PSUM bank: 8 transposes, one copy
            for kk in range(0, KPC, 2):
                k = ch * KPC + kk
                tps = tps_pool.tile([P, 2, CB, P * TW], mm_dt, tag="tps")
                if TW == 1:
                    wv = tps[:]
                else:
                    wv = tps[:].rearrange("p u cb (n t) -> p u cb n t", t=TW)[
                        :, :, :, :, 0
                    ]
                for u in range(2):
                    for cb in range(CB):
                        nc.tensor.transpose(
                            wv[:, u, cb, :],
                            xfcch[:, cb, (kk + u) * P : (kk + u + 1) * P],
                            ident,
                        )
                dst = xfT[:, k : k + 2, :].rearrange("p u (cb n) -> p u cb n", n=P)
                # xfT copies: 50/50 in the load phase (DVE idle), mostly ACT
                # later (DVE carries the epilogue)
                copy_mod = 2 if b == 0 else 4
                if (ch * 2 + kk // 2) % copy_mod == 0:
                    nc.vector.tensor_copy(out=dst, in_=wv)
                else:
                    nc.scalar.copy(out=dst, in_=wv)
            # energy accumulation for this chunk's k-pairs
            for cb in range(CB):
                e_ps = st["eps"][cb]
                if DR:
                    for kk in range(0, KPC, 2):
                        k = ch * KPC + kk
                        nc.tensor.matmul(
                            e_ps[:],
                            xfT[:, k : k + 2, cb * P : (cb + 1) * P],
                            xfT[:, k : k + 2, :],
                            start=(k == 0),
                            stop=(k + 2 >= KB),
                            perf_mode=mybir.MatmulPerfMode.DoubleRow,
                        )
                else:
                    for kk in range(KPC):
                        k = ch * KPC + kk
                        nc.tensor.matmul(
                            e_ps[:],
                            xfT[:, k, cb * P : (cb + 1) * P],
                            xfT[:, k, :],
                            start=(k == 0),
                            stop=(k == KB - 1),
                        )

        def softmax(b):
            st = states[b]
            Pmat = pmat_pool.tile([P, CB, C], mm_dt, tag="pmat")
            rS = small.tile([P, CB], F32, tag="rS")
            for cb in range(CB):
                e_ps = st["eps"][cb]
                m = small.tile([P, 1], F32, tag="m")
                nc.vector.tensor_reduce(
                    out=m[:], in_=e_ps[:], axis=mybir.AxisListType.X,
                    op=mybir.AluOpType.min,
                )
                S = small.tile([P, 1], F32, tag="S")
                nc.scalar.activation(
                    out=Pmat[:, cb, :],
                    in_=e_ps[:],
                    func=mybir.ActivationFunctionType.Exp,
                    bias=m[:],
                    scale=-1.0,
                    accum_out=S[:],
                )
                nc.vector.reciprocal(out=rS[:, cb : cb + 1], in_=S[:])

            beta = small.tile([P, CB], F32, tag="beta")
            nc.vector.tensor_tensor(
                out=beta[:],
                in0=rS[:],
                in1=gamma_sb[:].to_broadcast((P, CB)),
                op=mybir.AluOpType.mult,
            )
            st["beta"] = beta

            # PT transposes grouped by source row-block ob so each group can
            # start as soon as exp(ob) lands (no wait for all four exps).
            PT = pt_pool.tile([P, CB, C], mm_dt, tag="pt")
            for ob in range(CB):
                tps = tps_pool.tile([P, CB, P * TW], mm_dt, tag="tps")
                wv, rv = tps_views(tps)
                for cb in range(CB):
                    nc.tensor.transpose(
                        wv[:, cb, :], Pmat[:, ob, cb * P : (cb + 1) * P], ident
                    )
                dst = PT[:, :, ob * P : (ob + 1) * P]
                if ob % 2 == 0:
                    nc.vector.tensor_copy(out=dst, in_=rv)
                else:
                    nc.scalar.copy(out=dst, in_=rv)
            st["PT"] = PT

        def mm2_chunk(b, nh):
            st = states[b]
            PT, beta = st["PT"], st["beta"]
            yv = y[b].rearrange("(ob p) n -> p ob n", p=P)
            nsl = slice(nh * NCH_SZ, (nh + 1) * NCH_SZ)
            yt = yt_pool.tile([P, CB, NCH_SZ], BF16, tag="yt")
            for ob in range(CB):
                o_ps = ops_pool.tile([P, NCH_SZ], F32, tag="ops")
                if DR:
                    for cb in range(0, CB, 2):
                        nc.tensor.matmul(
                            o_ps[:],
                            PT[:, cb : cb + 2, ob * P : (ob + 1) * P],
                            st["xfc"][nh][:, cb : cb + 2, :],
                            start=(cb == 0),
                            stop=(cb + 2 >= CB),
                            perf_mode=mybir.MatmulPerfMode.DoubleRow,
                        )
                else:
                    for cb in range(CB):
                        nc.tensor.matmul(
                            o_ps[:],
                            PT[:, cb, ob * P : (ob + 1) * P],
                            st["xfc"][nh][:, cb, :],
                            start=(cb == 0),
                            stop=(cb == CB - 1),
                        )
                nc.vector.scalar_tensor_tensor(
                    out=yt[:, ob, :],
                    in0=o_ps[:],
                    scalar=beta[:, ob : ob + 1],
                    in1=st["xf"][nh][:, ob, :],
                    op0=mybir.AluOpType.mult,
                    op1=mybir.AluOpType.add,
                )
            # SWDGE so writes don't block the next sample's loads in the
            # HWDGE FIFO (gpsimd engine is otherwise idle)
            nc.gpsimd.dma_start(yv[:, :, nsl], yt[:])

        for ch in range(NCH):
            prefetch_chunk(0, ch)
        for b in range(BPC):
            if b + 1 < BPC:
                # hoist the next sample's first loads (DMA only) so they
                # queue right behind this sample's loads on the sync FIFO
                for ch in range(min(4, NCH)):
                    load_chunk(b + 1, ch)
            softmax(b)
            for nh in range(NCH):
                mm2_chunk(b, nh)
                if b + 1 < BPC:
                    prefetch_chunk(b + 1, nh)

    nc.finalize()
    return nc


def kernel(x: np.ndarray, gamma: np.ndarray) -> np.ndarray:
    global LAST_EXEC_TIME_NS, LAST_TRACE, LAST_PROFILE_JSON
    from concourse.bass_utils import run_bass_kernel_spmd

    assert x.shape == (B, C, H, W), x.shape
    x = np.ascontiguousarray(x, dtype=np.float32)
    gamma = np.ascontiguousarray(gamma, dtype=np.float32).reshape(1)

    name = MM_DT_NAME
    if name not in _CACHE:
        _CACHE[name] = _build(name)
    nc = _CACHE[name]

    xs = x.reshape(N_CORES, BPC, C, N)
    in_maps = [{"x": xs[i], "gamma": gamma} for i in range(N_CORES)]
    trace = os.environ.get("CAM_TRACE", "0") == "1"
    kwargs = {}
    if trace:
        import tempfile

        tmpdir = tempfile.mkdtemp(prefix=f"cam_trace_{name}_")
        try:
            os.unlink(f"/tmp/cam_trace_{name}")
        except OSError:
            pass
        os.symlink(tmpdir, f"/tmp/cam_trace_{name}")
        kwargs["tmpdir"] = tmpdir
    res = run_bass_kernel_spmd(
        nc, in_maps, core_ids=list(range(N_CORES)), trace=trace, **kwargs
    )
    LAST_EXEC_TIME_NS = res.exec_time_ns
    LAST_TRACE = res.instructions_and_trace
    LAST_PROFILE_JSON = res.profile_json
    out = np.concatenate(
        [np.asarray(res.results[i]["y"]).astype(np.float32) for i in range(N_CORES)],
        axis=0,
    )
    return out.reshape(B, C, H, W)

